# revision 15
# baseline (speedup 1.0000x reference)
"""Multi-head attention (B=2, P=2048, DIM=1024, H=16, d=64) on 8 trn2 cores.

Sharding: core c = 4*b + g handles batch b = c//4 and heads 4g..4g+3 (g = c%4).
Per core:
  - QKV projection for its 4 heads, computed in transposed layout
    (Q^T, K^T: [dh, seq]) directly off x^T (host pre-transposes x).
  - Attention per head in S^T orientation: S^T tiles [128k, 512q],
    exp on ScalarE (scale 1/8 folded), AV matmul with V augmented by a ones
    column (M=65) so the softmax denominator lands in PSUM row 64.
    Normalize with DVE reciprocal + gpsimd partition_broadcast.
  - AllToAll over all 8 cores exchanges O^T q-slices (two calls, one per
    local head-pair, for comm/compute overlap). Cross-batch shards are
    neutralized by zero rows in the host-prepared, permuted W_proj.
  - Output projection over the gathered [2048 x 512] O^T (8 real +
    8 zero dh-chunks) + bias; each core emits its [512, 1024] output slice.
"""

import sys

sys.path.insert(0, "/opt/trn_rl_repo")

import numpy as np
import concourse.bass as bass
import concourse.tile as tile
import concourse.mybir as mybir
from concourse import bacc
from concourse.bass import ts
from concourse.bass_utils import run_bass_kernel_spmd

FP = mybir.dt.float32
N_CORES = 8
B, P, DIM, H, D = 2, 2048, 1024, 16, 64
HPC = H // 4  # heads per core = 4
DHC = HPC * D  # dh per core = 256
QS = P // 4  # per-core q-slice = 512
NQ = P // 512  # 4 q-chunks of 512
NK = P // 128  # 16 k-chunks of 128
ND = DIM // 128  # 8 dim-chunks
EXP_GROUP = 3  # k-chunks per exp group (psum tile banks)
MM_DT = mybir.dt.bfloat16  # matmul operand dtype (1 cyc/row, half the DMA bytes)
EX_DT = mybir.dt.bfloat16  # exp output / AV moving operand dtype


def _mm(ap):
    return ap  # tiles feeding matmuls are allocated as MM_DT directly

_CACHE = {}


def _build(repeat=1, stop_after=None, fake_cc=False, n_cc=2):
    nc = bacc.Bacc(
        "TRN2",
        target_bir_lowering=False,
        debug=False,
        enable_asserts=False,
        num_devices=N_CORES,
    )
    xt = nc.dram_tensor("xt", [DIM, P], MM_DT, kind="ExternalInput").ap()
    wq = nc.dram_tensor("wq", [DIM, DHC], MM_DT, kind="ExternalInput").ap()
    wk = nc.dram_tensor("wk", [DIM, DHC], MM_DT, kind="ExternalInput").ap()
    wv = nc.dram_tensor("wv", [DIM, DHC], MM_DT, kind="ExternalInput").ap()
    wp = nc.dram_tensor("wp", [2 * DIM, DIM], MM_DT, kind="ExternalInput").ap()
    bias = nc.dram_tensor("bias", [128, DIM], FP, kind="ExternalInput").ap()
    out = nc.dram_tensor("out", [QS, DIM], FP, kind="ExternalOutput").ap()

    with tile.TileContext(nc) as tc:
        with (
            tc.tile_pool(name="s1", bufs=1) as s1,
            tc.tile_pool(name="es", bufs=7) as es,
            tc.tile_pool(name="wk2", bufs=2) as wk2,
            tc.tile_pool(name="dram", bufs=1, space="DRAM") as dram,
            tc.tile_pool(name="spool", bufs=2, space="PSUM") as spool,
            tc.tile_pool(name="avpool", bufs=2, space="PSUM") as avpool,
        ):
            qt_s = s1.tile([128, 2, P], MM_DT)
            kt_s = s1.tile([128, 2, NK, 128], MM_DT)
            v_s = s1.tile([128, NK, HPC, D + 1], EX_DT)
            bias_s = s1.tile([128, DIM], FP)
            nc.sync.dma_start(bias_s[:], bias[:])
            nc.vector.memset(v_s[:, :, :, D : D + 1], 1.0)

            # A2A buffers, slot-major. n_cc=4: [head-half, slot, 64, 512]
            # per pair; n_cc=2: [slot, half, 64, 512] per pair; n_cc=1:
            # [slot, head, 64, 512].
            if n_cc == 4:
                cc_in = [
                    dram.tile([2, 8, 64, QS], EX_DT, name=f"cci{j}") for j in range(2)
                ]
                cc_out = [
                    dram.tile([2, 8, 64, QS], EX_DT, name=f"cco{j}") for j in range(2)
                ]
            elif n_cc == 2:
                cc_in = [
                    dram.tile([8, 2, 64, QS], EX_DT, name=f"cci{j}") for j in range(2)
                ]
                cc_out = [
                    dram.tile([8, 2, 64, QS], EX_DT, name=f"cco{j}") for j in range(2)
                ]
            else:
                cc_in = dram.tile([8, 4, 64, QS], EX_DT, name="cci")
                cc_out = dram.tile([8, 4, 64, QS], EX_DT, name="cco")

            # ---- phase 1: QKV projection (ld pool closes afterwards) ------
            def one_pass():
              with tc.tile_pool(name="ld", bufs=1) as ld:
                xt_s = ld.tile([128, ND, P], MM_DT)
                wq_s = ld.tile([128, ND, DHC], MM_DT)
                wk_s = ld.tile([128, ND, DHC], MM_DT)
                wv_s = ld.tile([128, ND, DHC], MM_DT)
                def load_xt_block(qc):
                    for dc in range(ND):
                        nc.sync.dma_start(
                            xt_s[:, dc, ts(qc, 512)],
                            xt[ts(dc, 128), ts(qc, 512)],
                        )

                nc.sync.dma_start(wq_s[:], wq.rearrange("(c p) n -> p c n", p=128))
                load_xt_block(0)
                nc.sync.dma_start(wk_s[:], wk.rearrange("(c p) n -> p c n", p=128))
                nc.sync.dma_start(wv_s[:], wv.rearrange("(c p) n -> p c n", p=128))
                for qc in range(1, NQ):
                    load_xt_block(qc)

                def qk_chunk(j, qc):
                    psq = avpool.tile([128, 512], FP, tag="ps", name="psq")
                    psk = avpool.tile([128, 512], FP, tag="ps", name="psk")
                    for dc in range(ND):
                        nc.tensor.matmul(
                            psq[:],
                            _mm(wq_s[:, dc, ts(j, 128)]),
                            _mm(xt_s[:, dc, ts(qc, 512)]),
                            start=(dc == 0),
                            stop=(dc == ND - 1),
                        )
                    for dc in range(ND):
                        nc.tensor.matmul(
                            psk[:],
                            _mm(wk_s[:, dc, ts(j, 128)]),
                            _mm(xt_s[:, dc, ts(qc, 512)]),
                            start=(dc == 0),
                            stop=(dc == ND - 1),
                        )
                    nc.vector.tensor_copy(out=qt_s[:, j, ts(qc, 512)], in_=psq[:])
                    nc.vector.tensor_copy(
                        out=kt_s[:, j, 4 * qc : 4 * qc + 4, :],
                        in_=psk[:].rearrange("p (a b) -> p a b", b=128),
                    )

                def v_chunk(sc):
                    psv = avpool.tile([128, 512], FP, tag="ps", name="psv")
                    for dc in range(ND):
                        nc.tensor.matmul(
                            psv[:, 0:DHC],
                            _mm(xt_s[:, dc, ts(sc, 128)]),
                            _mm(wv_s[:, dc, :]),
                            start=(dc == 0),
                            stop=(dc == ND - 1),
                        )
                    nc.vector.tensor_copy(
                        out=v_s[:, sc, :, 0:D],
                        in_=psv[:, 0:DHC].rearrange("p (h d) -> p h d", d=D),
                    )

                for qc in range(NQ):
                    qk_chunk(0, qc)
                    for sc in range(4 * qc, 4 * qc + 4):
                        v_chunk(sc)

                # ---- round machinery (shared by both pool scopes) ---------
                groups = [
                    (k0, min(k0 + EXP_GROUP, NK)) for k0 in range(0, NK, EXP_GROUP)
                ]
                rg = [list(range(N_CORES))]
                late = {}  # og DMA + proj_pass, bound once the s2 pool exists
                deferred_og = []

                def emit_tail(h, qc, av):
                    j = h // 2
                    rec = wk2.tile([1, 512], FP, tag="rec", name="rec")
                    nc.vector.reciprocal(rec[:], av[D : D + 1, :])
                    bc = wk2.tile([64, 512], FP, tag="bc", name="bc")
                    nc.gpsimd.partition_broadcast(bc[:], rec[:])
                    om = wk2.tile([64, 512], EX_DT, tag="om", name="om")
                    nc.vector.tensor_mul(om[:], av[0:D, :], bc[:])
                    # slot i carries q-slice (i % 4); both batch groups get
                    # a copy (the other batch's is neutralized by zero wp)
                    if n_cc == 4:
                        nc.sync.dma_start(cc_in[j][h % 2, qc, :, :], om[:])
                        nc.sync.dma_start(cc_in[j][h % 2, qc + 4, :, :], om[:])
                    elif n_cc == 2:
                        nc.sync.dma_start(cc_in[j][qc, h % 2, :, :], om[:])
                        nc.sync.dma_start(cc_in[j][qc + 4, h % 2, :, :], om[:])
                    else:
                        nc.sync.dma_start(cc_in[qc, h, :, :], om[:])
                        nc.sync.dma_start(cc_in[qc + 4, h, :, :], om[:])

                def emit_cc(idx):
                    # A2A of 1, 2, or 4 heads' shard rows; splitting lets
                    # calls fire rounds earlier, merging amortizes the
                    # per-call collective overhead. og DMA may be deferred
                    # until the s2 pool exists.
                    if n_cc == 4:
                        j, half = divmod(idx, 2)
                        ci, co = cc_in[j][half], cc_out[j][half]
                    elif n_cc == 2:
                        ci, co = cc_in[idx][:], cc_out[idx][:]
                    else:
                        ci, co = cc_in[:], cc_out[:]
                    if fake_cc:
                        nc.sync.dma_start(co, ci)
                    else:
                        nc.gpsimd.collective_compute(
                            "AllToAll",
                            mybir.AluOpType.bypass,
                            replica_groups=rg,
                            ins=[ci.opt()],
                            outs=[co.opt()],
                        )
                    if "og" in late:
                        late["og"](idx)
                    else:
                        deferred_og.append(idx)

                import collections as _c

                pend = _c.deque()  # (h, av, ex, k0, k1, tail_info|None)

                def flush_one():
                    h_, av_, ex_, k0_, k1_, tinfo = pend.popleft()
                    for k in range(k0_, k1_):
                        nc.tensor.matmul(
                            av_[0 : D + 1, :],
                            _mm(v_s[:, k, h_, :]),
                            _mm(ex_[:, k - k0_, :]),
                            start=(k == 0),
                            stop=(k == NK - 1),
                            skip_group_check=True,
                        )
                    if tinfo is not None:
                        th, tqc = tinfo
                        emit_tail(th, tqc, av_)
                        if stop_after != "rounds" and tqc == NQ - 1:
                            if n_cc == 4 and th < 3:
                                emit_cc(2 * (th // 2) + th % 2)
                            elif n_cc == 2 and th == 1:
                                emit_cc(0)
                        if stop_after is None and th >= 2 and n_cc > 1:
                            rnd = (th - 2) * 4 + tqc
                            if rnd >= 2:
                                late["proj"](rnd - 2, 0, 8)

                av_cur = [None]

                def round_groups(h, qc, filler=None):
                    j, hp = h // 2, 64 * (h % 2)
                    for gi, (k0, k1) in enumerate(groups):
                        st = spool.tile(
                            [128, EXP_GROUP, 512], FP, tag="st", name="st"
                        )
                        for k in range(k0, k1):
                            nc.tensor.matmul(
                                st[:, k - k0, :],
                                _mm(kt_s[hp : hp + 64, j, k, :]),
                                _mm(qt_s[hp : hp + 64, j, ts(qc, 512)]),
                                start=True,
                                stop=True,
                            )
                        ex = es.tile(
                            [128, EXP_GROUP, 512], EX_DT, tag="ex", name="ex"
                        )
                        nc.scalar.activation(
                            out=ex[:, 0 : k1 - k0, :],
                            in_=st[:, 0 : k1 - k0, :],
                            func=mybir.ActivationFunctionType.Exp,
                            scale=float(D) ** -0.5,
                        )
                        if gi == 0:
                            av_cur[0] = avpool.tile(
                                [128, 512], FP, tag="ps", name="av"
                            )
                        pend.append(
                            (
                                h,
                                av_cur[0],
                                ex,
                                k0,
                                k1,
                                (h, qc) if gi == len(groups) - 1 else None,
                            )
                        )
                        while len(pend) > 2:
                            flush_one()
                    if filler is not None:
                        filler()

                def qk1_filler(qc):
                    # qk chunks for head-pair 1, squeezed into head-pair-0
                    # rounds. PSUM comes from the spool (freed by ACT, so no
                    # PE-order cycle with the in-flight AV accumulators).
                    stq = spool.tile([128, EXP_GROUP, 512], FP, tag="st", name="st")
                    for dc in range(ND):
                        nc.tensor.matmul(
                            stq[:, 0, :],
                            _mm(wq_s[:, dc, ts(1, 128)]),
                            _mm(xt_s[:, dc, ts(qc, 512)]),
                            start=(dc == 0),
                            stop=(dc == ND - 1),
                        )
                    for dc in range(ND):
                        nc.tensor.matmul(
                            stq[:, 1, :],
                            _mm(wk_s[:, dc, ts(1, 128)]),
                            _mm(xt_s[:, dc, ts(qc, 512)]),
                            start=(dc == 0),
                            stop=(dc == ND - 1),
                        )
                    nc.vector.tensor_copy(out=qt_s[:, 1, ts(qc, 512)], in_=stq[:, 0, :])
                    nc.vector.tensor_copy(
                        out=kt_s[:, 1, 4 * qc : 4 * qc + 4, :],
                        in_=stq[:, 1, :].rearrange("p (a b) -> p a b", b=128),
                    )

                # segment A: head-pair-0 rounds start as soon as j=0 QKV is
                # done; head-pair-1 QKV chunks ride in their PE idle
                if stop_after != "qkv":
                    for h in (0, 1):
                        for qc in range(NQ):
                            round_groups(
                                h,
                                qc,
                                filler=(
                                    (lambda q=qc: qk1_filler(q)) if h == 0 else None
                                ),
                            )

              if stop_after == "qkv":
                  nc.sync.dma_start(out[0:128, 0:256], qt_s[:, 0, 0:512].bitcast(FP))
                  return

              # ---- phase 2 + 3 (s2 reuses ld's sbuf range) ----------------
              with tc.tile_pool(name="s2", bufs=1) as s2:
                wp_s = s2.tile([128, 16, DIM], MM_DT)
                og_s = s2.tile([128, 16, QS], MM_DT)
                obuf = s2.tile([128, 8, 512], FP)
                nc.sync.dma_start(
                    wp_s[:], wp.rearrange("(c p) n -> p c n", p=128)
                )

                def og_dma(idx):
                    # og chunk 8j+s holds head 2j (partitions 0:64) + head
                    # 2j+1 (64:128) from sender slot s — same layout for all
                    # n_cc variants (wp host prep is invariant).
                    if n_cc == 4:
                        j, half = divmod(idx, 2)
                        nc.sync.dma_start(
                            og_s[64 * half : 64 * half + 64, 8 * j : 8 * j + 8, :],
                            cc_out[j][half].rearrange("s p n -> p s n"),
                        )
                    elif n_cc == 2:
                        nc.sync.dma_start(
                            og_s[:, 8 * idx : 8 * idx + 8, :],
                            cc_out[idx][:].rearrange("s i p n -> (i p) s n"),
                        )
                    else:
                        nc.sync.dma_start(
                            og_s[:],
                            cc_out[:].rearrange("s (c i) p n -> (i p) (c s) n", c=2),
                        )

                def proj_pass(u, c0, c1):
                    # output projection for (oc, sc) = divmod(u, 4), over
                    # gathered dh-chunks [c0:c1); two passes let chunks 0-7
                    # (ready after the early A2As) run inside round idle
                    oc, sc = divmod(u, 4)
                    pso = avpool.tile([128, 512], FP, tag="ps", name="pso")
                    for c in range(c0, c1):
                        nc.tensor.matmul(
                            pso[:],
                            _mm(og_s[:, c, ts(sc, 128)]),
                            _mm(wp_s[:, c, ts(oc, 512)]),
                            start=(c == c0),
                            stop=(c == c1 - 1),
                        )
                    if c0 == 0:
                        nc.vector.tensor_add(
                            obuf[:, u, :], pso[:], bias_s[:, ts(oc, 512)]
                        )
                    else:
                        nc.vector.tensor_add(obuf[:, u, :], pso[:], obuf[:, u, :])
                    if c1 == 16:
                        nc.sync.dma_start(
                            out[ts(sc, 128), ts(oc, 512)], obuf[:, u, :]
                        )

                late["og"] = og_dma
                late["proj"] = proj_pass
                for idx_ in deferred_og:
                    og_dma(idx_)

                # segment B: head-pair-1 rounds (+ pass-A proj injection)
                for h in (2, 3):
                    for qc in range(NQ):
                        round_groups(h, qc)
                while pend:
                    flush_one()
                if stop_after != "rounds":
                    emit_cc({4: 3, 2: 1, 1: 0}[n_cc])
                if stop_after == "rounds":
                    return
                if stop_after == "cc":
                    nc.sync.dma_start(out[0:128, 0:256], og_s[:, 0, :].bitcast(FP))
                    nc.sync.dma_start(out[128:256, 0:256], og_s[:, 8, :].bitcast(FP))
                    return

                # ---- phase 3: output projection (tail remainder) ----------
                if n_cc > 1:
                    for u in (6, 7):
                        proj_pass(u, 0, 8)
                    for u in range(8):
                        proj_pass(u, 8, 16)
                else:
                    for u in range(8):
                        proj_pass(u, 0, 16)

            for _rep in range(repeat):
                one_pass()

    nc.compile()
    return nc


def _prep_inputs(x, W_qkv, W_proj, b_proj):
    """Host-side sharding: per-core input dicts."""
    import ml_dtypes

    bf16 = ml_dtypes.bfloat16
    x = np.ascontiguousarray(np.asarray(x, dtype=np.float32))
    W_qkv = np.asarray(W_qkv, dtype=np.float32)
    W_proj = np.asarray(W_proj, dtype=np.float32)
    b_proj = np.asarray(b_proj, dtype=np.float32)

    bias_b = np.ascontiguousarray(np.broadcast_to(b_proj[None, :], (128, DIM)))
    in_maps = []
    for c in range(N_CORES):
        b, g = divmod(c, 4)
        xt = np.ascontiguousarray(x[b].T.astype(bf16))  # [DIM, P]
        wq = np.ascontiguousarray(W_qkv[:, 0 * DIM + DHC * g : 0 * DIM + DHC * (g + 1)].astype(bf16))
        wk = np.ascontiguousarray(W_qkv[:, 1 * DIM + DHC * g : 1 * DIM + DHC * (g + 1)].astype(bf16))
        wv = np.ascontiguousarray(W_qkv[:, 2 * DIM + DHC * g : 2 * DIM + DHC * (g + 1)].astype(bf16))
        # wp rows: [call a (head-pair 0), call b (pair 1)] x [slot s=0..7] x
        # [2 heads x 64]; slot s = sender rank s, holding heads 4*(s%4)+2a+i.
        # Slots from the other batch group are zeroed (their data is garbage
        # for this core).
        wp = np.zeros((2 * DIM, DIM), dtype=np.float32)
        for a in range(2):
            for s in range(8):
                if s // 4 != b:
                    continue
                for i in range(2):
                    h = 4 * (s % 4) + 2 * a + i
                    r0 = a * DIM + s * 128 + i * 64
                    wp[r0 : r0 + 64, :] = W_proj[64 * h : 64 * h + 64, :]
        in_maps.append(
            {"xt": xt, "wq": wq, "wk": wk, "wv": wv, "wp": wp.astype(bf16), "bias": bias_b}
        )
    return in_maps


def kernel(x, W_qkv, W_proj, b_proj, _trace=False, _tmpdir=None):
    if "nc" not in _CACHE:
        _CACHE["nc"] = _build()
    nc = _CACHE["nc"]
    in_maps = _prep_inputs(x, W_qkv, W_proj, b_proj)
    res = run_bass_kernel_spmd(
        nc,
        in_maps,
        core_ids=list(range(N_CORES)),
        trace=_trace,
        tmpdir=_tmpdir,
        stitch_traces=False,
    )
    _CACHE["last_results"] = res
    full = np.empty((B, P, DIM), dtype=np.float32)
    for c in range(N_CORES):
        b, g = divmod(c, 4)
        full[b, QS * g : QS * (g + 1), :] = res.results[c]["out"]
    return full

